# revision 52
# baseline (speedup 1.0000x reference)
"""Trainium2 Bass kernel for a Gaussian-routed top-2 MoE layer.

Strategy
--------
* Host (numpy): replicate the reference routing numerics exactly (fp32,
  XLA-CPU exp underflow semantics) to derive the top-2 dispatch and the
  int32 ``top_indices`` output, and to build per-core gather/pack inputs.
* Device (8 NeuronCores, SPMD):
  - routing phase: each core computes Gaussian log-probs + softmax weights
    for its 1/8 slice of tokens (fp32 matmuls via the quadratic expansion).
  - MoE phase: the 16384 (token, expert) pairs are packed into 8*S slots of
    T 128-token tiles (one expert per slot, zero-padded), S slots per core.
    Each slot runs x @ W1 -> gelu -> @ W2 with bf16 operands and fp32
    accumulation, multiplied by the softmax gate on-device.
* Host: scatter-free combine (each token's two gated expert outputs are
  gathered by row index and summed).

All matmuls keep tokens as the moving operand so no on-device transposes
are needed: phase A computes h^T (H on partitions), phase B consumes h^T as
the stationary operand to produce natural-layout outputs.
"""

import contextlib
import os
import time

os.environ.setdefault("NEURON_RT_RESET_CORES", "1")

import numpy as np
import ml_dtypes

import concourse.bacc as bacc
import concourse.mybir as mybir
from concourse.tile import TileContext
from concourse.bass_utils import run_bass_kernel_spmd

# Problem shape (fixed by the task).
B, TSEQ, D, H, DO, E, TOPK = 4, 2048, 1024, 4096, 1024, 8, 2
N = B * TSEQ              # 8192 tokens
NCORES = 8
NTOK = N // NCORES        # tokens per core for the routing phase
KD = D // 128             # contraction chunks for layer 1 (8)
MH = H // 128             # H chunks (32)
NO = DO // 512            # output column blocks (2)

# XLA:CPU (Eigen) vectorized expf returns exactly 0 below this input; numpy
# produces subnormals instead.  Reproducing this is required to match the
# reference's top-k tie-breaking on fully-saturated softmax rows.
_XLA_EXP_ZERO = np.float32(-87.33655)

_F32 = mybir.dt.float32
_BF16 = mybir.dt.bfloat16
_AF = mybir.ActivationFunctionType
_ALU = mybir.AluOpType
_BF_NP = ml_dtypes.bfloat16


def _host_routing(x_flat, expert_mus, expert_log_sigmas):
    """Exact replication of the reference routing in fp32 numpy."""
    sig = np.exp(expert_log_sigmas).astype(np.float32)
    sls = np.sum(expert_log_sigmas, -1).astype(np.float32)
    lp = np.empty((N, E), np.float32)
    for s in range(0, N, 1024):
        diff = (x_flat[s : s + 1024, None, :] - expert_mus[None]) / sig[None]
        lp[s : s + 1024] = (
            -0.5 * np.sum(diff * diff, axis=-1, dtype=np.float32) - sls
        )
    m = lp.max(1, keepdims=True)
    sh = lp - m
    un = np.where(sh > _XLA_EXP_ZERO, np.exp(sh), np.float32(0))
    w = (un / un.sum(1, keepdims=True)).astype(np.float32)
    order = np.argsort(-w, axis=1, kind="stable")
    ti = np.ascontiguousarray(order[:, :TOPK]).astype(np.int32)
    return lp, w, ti


def _plan_slots(ti):
    """Pack per-expert token lists into per-segment slots.

    The SPMD program gives every core S segments; segment j holds Tlist[j]
    128-token tiles and runs one expert.  Globally there are NCORES slots of
    each segment size.  An exact DP finds the smallest per-core tile total.

    Returns (Tlist, slots) where slots[j][c] = (expert, token_index_array)
    for core c's segment j.
    """
    import functools
    import itertools

    tok_by_e = [np.where((ti == e).any(1))[0] for e in range(E)]
    tiles_e = [max(0, -(-len(t) // 128)) for t in tok_by_e]

    def feasible(sizes):
        @functools.lru_cache(maxsize=None)
        def dp(ei, rem):
            if ei == E:
                return ()
            need = tiles_e[ei]
            maxa = [min(rem[j], need // sizes[j] + 1) for j in range(len(sizes))]
            for alloc in itertools.product(*[range(m + 1) for m in maxa]):
                if sum(a * s for a, s in zip(alloc, sizes)) >= need:
                    nr = tuple(r - a for r, a in zip(rem, alloc))
                    tail = dp(ei + 1, nr)
                    if tail is not None:
                        return (alloc,) + tail
            return None
        return dp(0, tuple([NCORES] * len(sizes)))

    # A slot's h^T tile must fit SBUF: cap per-segment tile counts at 15.
    TCAP = 15
    best = None
    for nseg in (1, 2, 3, 4):
        found = None
        for total in range(max(1, -(-sum(tiles_e) // NCORES)), 64):
            if nseg == 1:
                combos = [(total,)]
            elif nseg == 2:
                combos = [(a, total - a) for a in range(1, total)
                          if a >= total - a > 0]
            elif nseg == 3:
                combos = [(a, b, total - a - b)
                          for a in range(1, total) for b in range(1, total)
                          if a >= b >= total - a - b > 0]
            else:
                combos = [(a, b, c, total - a - b - c)
                          for a in range(1, total) for b in range(1, total)
                          for c in range(1, total)
                          if a >= b >= c >= total - a - b - c > 0]
            for sizes in combos:
                if max(sizes) > TCAP:
                    continue
                allocs = feasible(sizes)
                if allocs is not None:
                    found = (total, sizes, allocs)
                    break
            if found:
                break
        if found is None:
            continue
        total, sizes, allocs = found
        # cost: padded compute + per-segment weight streaming / fixed costs
        cost = total * 128 * 0.30 + nseg * 18.0
        if best is None or cost < best[0]:
            best = (cost, sizes, allocs)
    _, sizes, allocs = best
    S = len(sizes)

    # Materialize slot assignments: NCORES slots per segment index.
    slots = [[None] * NCORES for _ in range(S)]
    nextslot = [0] * S
    for e in range(E):
        toks = tok_by_e[e]
        off = 0
        for j in range(S):
            for _ in range(allocs[e][j]):
                take = toks[off : off + sizes[j] * 128]
                off += len(take)
                slots[j][nextslot[j]] = (e, take)
                nextslot[j] += 1
        assert off == len(toks), (e, off, len(toks))
    for j in range(S):
        for c in range(NCORES):
            if slots[j][c] is None:
                slots[j][c] = (0, np.zeros(0, np.int64))
    return list(sizes), slots


def _blocks(T):
    """Split T tiles into the fewest token blocks of <= 512 columns."""
    nb = -(-T * 128 // 512)
    base, rem = divmod(T, nb)
    out, off = [], 0
    for i in range(nb):
        blen = (base + (1 if i < rem else 0)) * 128
        out.append((off, blen))
        off += blen
    return out


def _null_ctx():
    return contextlib.nullcontext(None)


_KERNEL_CACHE = {}


def _build_kernel(Tlist, routing_last=False, pa_bufs=3, pb_bufs=3,
                  ob_bufs=4, w1_bufs=3, w2_bufs=36, xg_bufs=2, ht_bufs=1,
                  parts="both", uniform_a=False, tail_seg=None):
    """Build the SPMD Bass program for a per-segment tile plan ``Tlist``."""
    Tlist = tuple(Tlist)
    key = (Tlist, routing_last, pa_bufs, pb_bufs, ob_bufs, w1_bufs, w2_bufs,
           xg_bufs, ht_bufs, parts, uniform_a, tail_seg)
    if key in _KERNEL_CACHE:
        return _KERNEL_CACHE[key]

    S = len(Tlist)
    TT = sum(Tlist)          # total tiles per core
    ST = TT * 128            # MoE tokens per core
    seg_tok0 = [128 * sum(Tlist[:j]) for j in range(S)]   # token offsets
    seg_t0 = [sum(Tlist[:j]) for j in range(S)]           # tile offsets

    nc = bacc.Bacc("TRN2", target_bir_lowering=False, debug=False,
                   num_devices=NCORES)

    xT = nc.dram_tensor("xT", [D, NTOK], _F32, kind="ExternalInput")
    AM = nc.dram_tensor("AM", [D, 2 * E], _F32, kind="ExternalInput")
    crow = nc.dram_tensor("crow", [1, E], _F32, kind="ExternalInput")
    if uniform_a:
        # x in natural layout + the common -0.5/sigma^2 value (replicated).
        xn = nc.dram_tensor("xn", [NTOK, D], _F32, kind="ExternalInput")
        a0h = nc.dram_tensor("a0h", [128, 1], _F32, kind="ExternalInput")
    xgT = nc.dram_tensor("xgT", [D, ST], _BF16, kind="ExternalInput")
    w1 = nc.dram_tensor("w1", [S, KD, MH, 128, 128], _BF16, kind="ExternalInput")
    w2 = nc.dram_tensor("w2", [S, MH, NO, 128, 512], _BF16, kind="ExternalInput")
    b1t = nc.dram_tensor("b1t", [S, 128, MH], _F32, kind="ExternalInput")
    gT = nc.dram_tensor("gT", [128, TT], _F32, kind="ExternalInput")

    lp_out = nc.dram_tensor("lp_out", [NTOK, E], _F32, kind="ExternalOutput")
    w_out = nc.dram_tensor("w_out", [NTOK, E], _F32, kind="ExternalOutput")
    out = nc.dram_tensor("out", [ST, DO], _F32, kind="ExternalOutput")

    with TileContext(nc) as tc:
        with tc.tile_pool(name="const", bufs=1) as cp:
            ones_f = cp.tile([1, 128], _F32, tag="ones_f")
            nc.vector.memset(ones_f[:], 1.0)

            def emit_routing(on_the_fly_sq=False):
                with tc.tile_pool(name="rt", bufs=1) as rp, \
                     tc.tile_pool(name="rtk", bufs=KD) as rpk, \
                     tc.tile_pool(name="rsq", bufs=6) as rsq, \
                     tc.tile_pool(name="rxn", bufs=2) as rxn, \
                     tc.tile_pool(name="rtp", bufs=2, space="PSUM") as rpp, \
                     tc.tile_pool(name="rts", bufs=3) as rsp:
                    am_sb = rp.tile([128, KD * 2 * E], _F32, tag="am")
                    nc.sync.dma_start(
                        am_sb[:], AM.ap().rearrange("(k p) c -> p k c", p=128))
                    c_sb = rp.tile([1, E], _F32, tag="c")
                    nc.sync.dma_start(c_sb[:], crow[:, :])
                    xt_k, x2_k = [], []
                    xt_src = xT.ap().rearrange("(k p) t -> k p t", p=128)
                    for k in range(KD):
                        xt = rpk.tile([128, NTOK], _F32, tag="xt")
                        nc.sync.dma_start(xt[:], xt_src[k])
                        xt_k.append(xt)
                        if not (on_the_fly_sq or uniform_a):
                            x2 = rpk.tile([128, NTOK], _F32, tag="x2")
                            nc.scalar.activation(x2[:], xt[:], _AF.Square)
                            x2_k.append(x2)
                    if uniform_a:
                        a0_sb = rp.tile([128, 1], _F32, tag="a0")
                        nc.sync.dma_start(a0_sb[:], a0h[:, :])

                    for t0 in range(0, NTOK, 128):
                        pa = rpp.tile([128, E], _F32, tag="pa")
                        nc.tensor.matmul(pa[:], ones_f[:], c_sb[:],
                                         start=True, stop=False)
                        if uniform_a:
                            # sum(x^2)*a0 per token via ACT accumulate.
                            xn_sb = rxn.tile([128, D], _F32, tag="xn")
                            nc.sync.dma_start(xn_sb[:], xn[t0 : t0 + 128, :])
                            sq = rxn.tile([128, D], _F32, tag="sq")
                            s2 = rsp.tile([128, 1], _F32, tag="s2")
                            nc.scalar.activation(sq[:], xn_sb[:], _AF.Square,
                                                 accum_out=s2[:])
                            bn = rsp.tile([128, 1], _F32, tag="bn")
                            nc.vector.tensor_scalar_mul(bn[:], s2[:],
                                                        a0_sb[:])
                        else:
                            for k in range(KD):
                                if on_the_fly_sq:
                                    x2s = rsq.tile([128, 128], _F32, tag="x2s")
                                    nc.scalar.activation(
                                        x2s[:], xt_k[k][:, t0 : t0 + 128],
                                        _AF.Square)
                                    x2ap = x2s[:]
                                else:
                                    x2ap = x2_k[k][:, t0 : t0 + 128]
                                nc.tensor.matmul(
                                    pa[:], x2ap,
                                    am_sb[:, k * 2 * E : k * 2 * E + E],
                                    start=False, stop=False)
                        for k in range(KD):
                            nc.tensor.matmul(
                                pa[:],
                                xt_k[k][:, t0 : t0 + 128],
                                am_sb[:, k * 2 * E + E : (k + 1) * 2 * E],
                                start=False, stop=(k == KD - 1))
                        lp_sb = rsp.tile([128, E], _F32, tag="lp")
                        if uniform_a:
                            nc.scalar.activation(lp_sb[:], pa[:],
                                                 _AF.Identity,
                                                 bias=bn[:], scale=-0.5)
                        else:
                            nc.scalar.activation(lp_sb[:], pa[:], _AF.Copy,
                                                 bias=0.0, scale=-0.5)
                        nc.sync.dma_start(lp_out[t0 : t0 + 128, :], lp_sb[:])
                        nmax = rsp.tile([128, 1], _F32, tag="nmax")
                        nc.vector.tensor_reduce(nmax[:], lp_sb[:],
                                                axis=mybir.AxisListType.X,
                                                op=_ALU.max, negate=True)
                        esb = rsp.tile([128, E], _F32, tag="esb")
                        ssb = rsp.tile([128, 1], _F32, tag="ssb")
                        nc.scalar.activation(esb[:], lp_sb[:], _AF.Exp,
                                             bias=nmax[:], scale=1.0,
                                             accum_out=ssb[:])
                        rsb = rsp.tile([128, 1], _F32, tag="rsb")
                        nc.vector.reciprocal(rsb[:], ssb[:])
                        wsb = rsp.tile([128, E], _F32, tag="wsb")
                        nc.vector.tensor_scalar_mul(wsb[:], esb[:], rsb[:])
                        nc.sync.dma_start(w_out[t0 : t0 + 128, :], wsb[:])

            def emit_moe(tail=None, tail_seg=None):
                with tc.tile_pool(name="xg", bufs=xg_bufs) as xgp, \
                     tc.tile_pool(name="ht", bufs=ht_bufs) as htp, \
                     tc.tile_pool(name="w1p", bufs=w1_bufs) as w1p, \
                     tc.tile_pool(name="w2p", bufs=w2_bufs) as w2p, \
                     tc.tile_pool(name="sm", bufs=2) as smp, \
                     tc.tile_pool(name="ob", bufs=ob_bufs) as obp, \
                     tc.tile_pool(name="pA", bufs=abs(pa_bufs), space="PSUM") as pap, \
                     tc.tile_pool(name="pB", bufs=pb_bufs, space="PSUM") as _pbx:
                    pbp = pap if pa_bufs < 0 else _pbx
                    for seg in range(S):
                        T = Tlist[seg]
                        TC = T * 128
                        xg_sb = xgp.tile([128, KD * TC], _BF16, tag="xg")
                        nc.sync.dma_start(
                            xg_sb[:],
                            xgT.ap().rearrange("(k p) t -> p k t", p=128)
                            [:, :, seg_tok0[seg] : seg_tok0[seg] + TC])
                        b1_sb = smp.tile([128, MH], _F32, tag="b1")
                        nc.sync.dma_start(b1_sb[:], b1t[seg])
                        g_sb = smp.tile([128, T], _F32, tag="g")
                        nc.sync.dma_start(
                            g_sb[:], gT[:, seg_t0[seg] : seg_t0[seg] + T])

                        ht_sb = htp.tile([128, MH * TC], _BF16, tag="ht")

                        # phase A: ht[m, tok] = gelu(W1^T x + b1)
                        for m in range(MH):
                            w1m = w1p.tile([128, KD * 128], _BF16, tag="w1m")
                            nc.sync.dma_start(
                                w1m[:],
                                w1[seg, :, m].rearrange("k p q -> p k q"))
                            for boff, blen in _blocks(T):
                                ps = pap.tile([128, blen], _F32, tag="pA")
                                for k in range(KD):
                                    nc.tensor.matmul(
                                        ps[:],
                                        w1m[:, k * 128 : (k + 1) * 128],
                                        xg_sb[:, k * TC + boff : k * TC + boff + blen],
                                        start=(k == 0), stop=(k == KD - 1))
                                nc.scalar.activation(
                                    ht_sb[:, m * TC + boff : m * TC + boff + blen],
                                    ps[:], _AF.Gelu,
                                    bias=b1_sb[:, m : m + 1], scale=1.0)

                        # phase B: out[tok, n] = (ht^T W2 + b2) * gate
                        for n in range(NO):
                            w2t = []
                            for k in range(MH):
                                wt = w2p.tile([128, 512], _BF16, tag="w2t")
                                nc.sync.dma_start(wt[:], w2[seg, k, n])
                                w2t.append(wt)
                            for t in range(T):
                                ps = pbp.tile([128, 512], _F32, tag="pA" if pa_bufs < 0 else "pB")
                                for k in range(MH):
                                    nc.tensor.matmul(
                                        ps[:],
                                        ht_sb[:, k * TC + t * 128 : k * TC + t * 128 + 128],
                                        w2t[k][:],
                                        start=(k == 0), stop=(k == MH - 1))
                                osb = obp.tile([128, 512], _F32, tag="osb")
                                nc.vector.tensor_scalar_mul(
                                    osb[:], ps[:], g_sb[:, t : t + 1])
                                nc.sync.dma_start(
                                    out[(seg_t0[seg] + t) * 128 : (seg_t0[seg] + t + 1) * 128,
                                        n * 512 : (n + 1) * 512],
                                    osb[:])
                        if tail is not None and seg == tail_seg:
                            tail()
                    if tail is not None and tail_seg is None:
                        tail()

            if parts == "moe":
                emit_moe()
            elif parts == "routing":
                emit_routing()
            elif parts == "nested":
                emit_moe(tail=lambda: emit_routing(on_the_fly_sq=True),
                         tail_seg=tail_seg)
            elif routing_last:
                emit_moe()
                emit_routing()
            else:
                emit_routing()
                emit_moe()

    nc.finalize()
    _KERNEL_CACHE[key] = nc
    return nc


def _build_inputs(x_flat, W1, b1, W2, b2, expert_mus, expert_log_sigmas,
                  w_host, Tlist, slots, uniform_a=False):
    """Per-core input dicts + the (locA, locB) combine rows."""
    sig2 = np.exp(expert_log_sigmas).astype(np.float32) ** 2
    a = (1.0 / sig2).astype(np.float32)                      # [E, D]
    m2 = (-2.0 * expert_mus * a).astype(np.float32)          # [E, D]
    cvec = np.sum(expert_mus * expert_mus * a, -1) + 2.0 * np.sum(
        expert_log_sigmas, -1)
    AM = np.concatenate([a.T, m2.T], axis=1).astype(np.float32)  # [D, 2E]
    crow = cvec.reshape(1, E).astype(np.float32)

    S = len(Tlist)
    TT = sum(Tlist)
    ST = TT * 128
    seg_tok0 = [128 * sum(Tlist[:j]) for j in range(S)]
    seg_t0 = [sum(Tlist[:j]) for j in range(S)]
    xT_all = np.ascontiguousarray(x_flat.T)                  # [D, N]
    w1_bf = W1.astype(_BF_NP)
    w2_bf = W2.astype(_BF_NP)

    in_maps = []
    tok_all, row_all = [], []
    for c in range(NCORES):
        xgT = np.zeros((D, ST), _BF_NP)
        gTm = np.zeros((128, TT), np.float32)
        w1g = np.empty((S, KD, MH, 128, 128), _BF_NP)
        w2g = np.empty((S, MH, NO, 128, 512), _BF_NP)
        b1g = np.empty((S, 128, MH), np.float32)
        for s in range(S):
            T = Tlist[s]
            TC = T * 128
            e, toks = slots[s][c]
            nt = len(toks)
            if nt:
                xgT[:, seg_tok0[s] : seg_tok0[s] + nt] = \
                    xT_all[:, toks].astype(_BF_NP)
                g = w_host[toks, e]
                gfull = np.zeros(TC, np.float32)
                gfull[:nt] = g
                gTm[:, seg_t0[s] : seg_t0[s] + T] = gfull.reshape(T, 128).T
                row0 = c * ST + seg_tok0[s]
                tok_all.append(toks)
                row_all.append(row0 + np.arange(nt, dtype=np.int64))
            w1g[s] = (
                w1_bf[e]
                .reshape(KD, 128, MH, 128).transpose(0, 2, 1, 3))
            w2g[s] = (
                w2_bf[e]
                .reshape(MH, 128, NO, 512).transpose(0, 2, 1, 3))
            b1g[s] = b1[e].reshape(MH, 128).T
        in_map = {
            "xT": np.ascontiguousarray(xT_all[:, c * NTOK : (c + 1) * NTOK]),
            "AM": AM, "crow": crow,
            "xgT": xgT,
            "w1": np.ascontiguousarray(w1g),
            "w2": np.ascontiguousarray(w2g),
            "b1t": b1g, "gT": gTm,
        }
        if uniform_a:
            in_map["xn"] = np.ascontiguousarray(
                x_flat[c * NTOK : (c + 1) * NTOK])
            in_map["a0h"] = np.full((128, 1), -0.5 * a.flat[0], np.float32)
        in_maps.append(in_map)

    tok_all = np.concatenate(tok_all)
    row_all = np.concatenate(row_all)
    order = np.argsort(tok_all, kind="stable")
    assert (tok_all[order][0::2] == np.arange(N)).all()
    assert (tok_all[order][1::2] == np.arange(N)).all()
    locA = row_all[order[0::2]]
    locB = row_all[order[1::2]]
    return in_maps, locA, locB


def kernel(x, expert_mus, expert_log_sigmas, W1, b1, W2, b2):
    x = np.asarray(x, np.float32)
    expert_mus = np.asarray(expert_mus, np.float32)
    expert_log_sigmas = np.asarray(expert_log_sigmas, np.float32)
    W1 = np.asarray(W1, np.float32)
    b1 = np.asarray(b1, np.float32)
    W2 = np.asarray(W2, np.float32)
    b2 = np.asarray(b2, np.float32)

    x_flat = np.ascontiguousarray(x.reshape(N, D))
    lp_host, w_host, ti = _host_routing(x_flat, expert_mus, expert_log_sigmas)
    Tlist, slots = _plan_slots(ti)
    # Uniform-sigma fast path: x^2 routing term reduces to a per-token
    # scalar computable on the ACT engine, halving routing matmuls.
    a_all = np.exp(-2.0 * expert_log_sigmas).astype(np.float32)
    uniform_a = bool((a_all == a_all.flat[0]).all())
    # Adaptive SBUF budget (KB/partition, ~186 usable): ht=8*Tmax,
    # xg=2*Tmax per buf, w1=2 per buf, w2=1 per buf, ob=8, routing~36-52.
    Tmax = max(Tlist)
    routing_kb = 50 if uniform_a else 36
    w2_nested = int(min(52, 196 - routing_kb - 10 * Tmax - 18))
    xg_bufs = 2 if 8 * Tmax + 2 * 2 * Tmax + 80 <= 186 else 1
    w2_bufs = int(min(56, max(8, 186 - 8 * Tmax - xg_bufs * 2 * Tmax - 25)))
    cfgs = []
    if w2_nested >= 34:
        # Routing overlaps the MoE phase (shares SBUF budget with it).
        cfgs.append(dict(parts="nested", w1_bufs=6, w2_bufs=w2_nested,
                         xg_bufs=1, uniform_a=uniform_a))
        cfgs.append(dict(parts="nested", w1_bufs=4,
                         w2_bufs=max(34, w2_nested - 8),
                         xg_bufs=1, uniform_a=uniform_a))
    cfgs.append(dict(routing_last=True,
                     w1_bufs=6 if w2_bufs >= 24 else 3, w2_bufs=w2_bufs,
                     xg_bufs=xg_bufs, uniform_a=uniform_a))
    nc = None
    for cfg in cfgs:
        try:
            nc = _build_kernel(Tlist, **cfg)
            break
        except Exception:  # noqa: BLE001 - SBUF/PSUM overflow: try smaller
            if cfg is cfgs[-1]:
                raise
    in_maps, locA, locB = _build_inputs(
        x_flat, W1, b1, W2, b2, expert_mus, expert_log_sigmas,
        w_host, Tlist, slots, uniform_a=uniform_a)

    res = None
    for attempt in range(3):
        try:
            res = run_bass_kernel_spmd(nc, in_maps,
                                       core_ids=list(range(NCORES)))
            break
        except Exception:  # noqa: BLE001 - transient NRT/device wedges
            if attempt == 2:
                raise
            time.sleep(5.0)

    lp = np.concatenate([r["lp_out"] for r in res.results]).reshape(B, TSEQ, E)
    w = np.concatenate([r["w_out"] for r in res.results]).reshape(B, TSEQ, E)
    outcat = np.concatenate([r["out"] for r in res.results], axis=0)
    # The device computes gate * (h @ W2); the gate * b2 term is added here.
    gsel = np.take_along_axis(w_host, ti, axis=1)            # [N, K]
    bias = np.einsum("nk,nkd->nd", gsel, b2[ti])             # [N, DO]
    final = (outcat[locA] + outcat[locB] + bias).reshape(B, TSEQ, DO)
    routing_info = {"log_probs": lp, "weights": w, "top_indices": ti}
    return final, routing_info


# revision 54
# speedup vs baseline: 1.0041x; 1.0041x over previous
"""Trainium2 Bass kernel for a Gaussian-routed top-2 MoE layer.

Strategy
--------
* Host (numpy): replicate the reference routing numerics exactly (fp32,
  XLA-CPU exp underflow semantics) to derive the top-2 dispatch and the
  int32 ``top_indices`` output, and to build per-core gather/pack inputs.
* Device (8 NeuronCores, SPMD):
  - routing phase: each core computes Gaussian log-probs + softmax weights
    for its 1/8 slice of tokens (fp32 matmuls via the quadratic expansion).
  - MoE phase: the 16384 (token, expert) pairs are packed into 8*S slots of
    T 128-token tiles (one expert per slot, zero-padded), S slots per core.
    Each slot runs x @ W1 -> gelu -> @ W2 with bf16 operands and fp32
    accumulation, multiplied by the softmax gate on-device.
* Host: scatter-free combine (each token's two gated expert outputs are
  gathered by row index and summed).

All matmuls keep tokens as the moving operand so no on-device transposes
are needed: phase A computes h^T (H on partitions), phase B consumes h^T as
the stationary operand to produce natural-layout outputs.
"""

import contextlib
import os
import time

os.environ.setdefault("NEURON_RT_RESET_CORES", "1")

import numpy as np
import ml_dtypes

import concourse.bacc as bacc
import concourse.mybir as mybir
from concourse.tile import TileContext
from concourse.bass_utils import run_bass_kernel_spmd

# Problem shape (fixed by the task).
B, TSEQ, D, H, DO, E, TOPK = 4, 2048, 1024, 4096, 1024, 8, 2
N = B * TSEQ              # 8192 tokens
NCORES = 8
NTOK = N // NCORES        # tokens per core for the routing phase
KD = D // 128             # contraction chunks for layer 1 (8)
MH = H // 128             # H chunks (32)
NO = DO // 512            # output column blocks (2)

# XLA:CPU (Eigen) vectorized expf returns exactly 0 below this input; numpy
# produces subnormals instead.  Reproducing this is required to match the
# reference's top-k tie-breaking on fully-saturated softmax rows.
_XLA_EXP_ZERO = np.float32(-87.33655)

_F32 = mybir.dt.float32
_BF16 = mybir.dt.bfloat16
_AF = mybir.ActivationFunctionType
_ALU = mybir.AluOpType
_BF_NP = ml_dtypes.bfloat16


def _host_routing(x_flat, expert_mus, expert_log_sigmas):
    """Exact replication of the reference routing in fp32 numpy."""
    sig = np.exp(expert_log_sigmas).astype(np.float32)
    sls = np.sum(expert_log_sigmas, -1).astype(np.float32)
    lp = np.empty((N, E), np.float32)
    for s in range(0, N, 1024):
        diff = (x_flat[s : s + 1024, None, :] - expert_mus[None]) / sig[None]
        lp[s : s + 1024] = (
            -0.5 * np.sum(diff * diff, axis=-1, dtype=np.float32) - sls
        )
    m = lp.max(1, keepdims=True)
    sh = lp - m
    un = np.where(sh > _XLA_EXP_ZERO, np.exp(sh), np.float32(0))
    w = (un / un.sum(1, keepdims=True)).astype(np.float32)
    order = np.argsort(-w, axis=1, kind="stable")
    ti = np.ascontiguousarray(order[:, :TOPK]).astype(np.int32)
    return lp, w, ti


def _plan_slots(ti):
    """Pack per-expert token lists into per-segment slots.

    The SPMD program gives every core S segments; segment j holds Tlist[j]
    128-token tiles and runs one expert.  Globally there are NCORES slots of
    each segment size.  An exact DP finds the smallest per-core tile total.

    Returns (Tlist, slots) where slots[j][c] = (expert, token_index_array)
    for core c's segment j.
    """
    import functools
    import itertools

    tok_by_e = [np.where((ti == e).any(1))[0] for e in range(E)]
    tiles_e = [max(0, -(-len(t) // 128)) for t in tok_by_e]

    def feasible(sizes):
        @functools.lru_cache(maxsize=None)
        def dp(ei, rem):
            if ei == E:
                return ()
            need = tiles_e[ei]
            maxa = [min(rem[j], need // sizes[j] + 1) for j in range(len(sizes))]
            for alloc in itertools.product(*[range(m + 1) for m in maxa]):
                if sum(a * s for a, s in zip(alloc, sizes)) >= need:
                    nr = tuple(r - a for r, a in zip(rem, alloc))
                    tail = dp(ei + 1, nr)
                    if tail is not None:
                        return (alloc,) + tail
            return None
        return dp(0, tuple([NCORES] * len(sizes)))

    # A slot's h^T tile must fit SBUF: cap per-segment tile counts at 15.
    TCAP = 15
    best = None
    for nseg in (1, 2, 3, 4):
        found = None
        for total in range(max(1, -(-sum(tiles_e) // NCORES)), 64):
            if nseg == 1:
                combos = [(total,)]
            elif nseg == 2:
                combos = [(a, total - a) for a in range(1, total)
                          if a >= total - a > 0]
            elif nseg == 3:
                combos = [(a, b, total - a - b)
                          for a in range(1, total) for b in range(1, total)
                          if a >= b >= total - a - b > 0]
            else:
                combos = [(a, b, c, total - a - b - c)
                          for a in range(1, total) for b in range(1, total)
                          for c in range(1, total)
                          if a >= b >= c >= total - a - b - c > 0]
            for sizes in combos:
                if max(sizes) > TCAP:
                    continue
                allocs = feasible(sizes)
                if allocs is not None:
                    found = (total, sizes, allocs)
                    break
            if found:
                break
        if found is None:
            continue
        total, sizes, allocs = found
        # cost: padded compute + per-segment weight streaming / fixed costs
        cost = total * 128 * 0.30 + nseg * 18.0
        if best is None or cost < best[0]:
            best = (cost, sizes, allocs)
    _, sizes, allocs = best
    # Smallest segment first: shortens the startup-critical first xg DMA.
    perm = sorted(range(len(sizes)), key=lambda j: sizes[j])
    sizes = tuple(sizes[j] for j in perm)
    allocs = tuple(tuple(al[j] for j in perm) for al in allocs)
    S = len(sizes)

    # Materialize slot assignments: NCORES slots per segment index.
    slots = [[None] * NCORES for _ in range(S)]
    nextslot = [0] * S
    for e in range(E):
        toks = tok_by_e[e]
        off = 0
        for j in range(S):
            for _ in range(allocs[e][j]):
                take = toks[off : off + sizes[j] * 128]
                off += len(take)
                slots[j][nextslot[j]] = (e, take)
                nextslot[j] += 1
        assert off == len(toks), (e, off, len(toks))
    for j in range(S):
        for c in range(NCORES):
            if slots[j][c] is None:
                slots[j][c] = (0, np.zeros(0, np.int64))
    return list(sizes), slots


def _blocks(T):
    """Split T tiles into the fewest token blocks of <= 512 columns."""
    nb = -(-T * 128 // 512)
    base, rem = divmod(T, nb)
    out, off = [], 0
    for i in range(nb):
        blen = (base + (1 if i < rem else 0)) * 128
        out.append((off, blen))
        off += blen
    return out


def _null_ctx():
    return contextlib.nullcontext(None)


_KERNEL_CACHE = {}


def _build_kernel(Tlist, routing_last=False, pa_bufs=3, pb_bufs=3,
                  ob_bufs=4, w1_bufs=3, w2_bufs=36, xg_bufs=2, ht_bufs=1,
                  parts="both", uniform_a=False, tail_seg=None,
                  rpp_bufs=2):
    """Build the SPMD Bass program for a per-segment tile plan ``Tlist``."""
    Tlist = tuple(Tlist)
    key = (Tlist, routing_last, pa_bufs, pb_bufs, ob_bufs, w1_bufs, w2_bufs,
           xg_bufs, ht_bufs, parts, uniform_a, tail_seg, rpp_bufs)
    if key in _KERNEL_CACHE:
        return _KERNEL_CACHE[key]

    S = len(Tlist)
    TT = sum(Tlist)          # total tiles per core
    ST = TT * 128            # MoE tokens per core
    seg_tok0 = [128 * sum(Tlist[:j]) for j in range(S)]   # token offsets
    seg_t0 = [sum(Tlist[:j]) for j in range(S)]           # tile offsets

    nc = bacc.Bacc("TRN2", target_bir_lowering=False, debug=False,
                   num_devices=NCORES)

    xT = nc.dram_tensor("xT", [D, NTOK], _F32, kind="ExternalInput")
    AM = nc.dram_tensor("AM", [D, 2 * E], _F32, kind="ExternalInput")
    crow = nc.dram_tensor("crow", [1, E], _F32, kind="ExternalInput")
    if uniform_a:
        # x in natural layout + the common -0.5/sigma^2 value (replicated).
        xn = nc.dram_tensor("xn", [NTOK, D], _F32, kind="ExternalInput")
        a0h = nc.dram_tensor("a0h", [128, 1], _F32, kind="ExternalInput")
    xgT = nc.dram_tensor("xgT", [D, ST], _BF16, kind="ExternalInput")
    w1 = nc.dram_tensor("w1", [S, KD, MH, 128, 128], _BF16, kind="ExternalInput")
    w2 = nc.dram_tensor("w2", [S, MH, NO, 128, 512], _BF16, kind="ExternalInput")
    b1t = nc.dram_tensor("b1t", [S, 128, MH], _F32, kind="ExternalInput")
    gT = nc.dram_tensor("gT", [128, TT], _F32, kind="ExternalInput")

    lp_out = nc.dram_tensor("lp_out", [NTOK, E], _F32, kind="ExternalOutput")
    w_out = nc.dram_tensor("w_out", [NTOK, E], _F32, kind="ExternalOutput")
    out = nc.dram_tensor("out", [ST, DO], _F32, kind="ExternalOutput")

    with TileContext(nc) as tc:
        with tc.tile_pool(name="const", bufs=1) as cp:
            ones_f = cp.tile([1, 128], _F32, tag="ones_f")
            nc.vector.memset(ones_f[:], 1.0)

            def emit_routing(on_the_fly_sq=False):
                with tc.tile_pool(name="rt", bufs=1) as rp, \
                     tc.tile_pool(name="rtk", bufs=KD) as rpk, \
                     tc.tile_pool(name="rsq", bufs=6) as rsq, \
                     tc.tile_pool(name="rxn", bufs=2) as rxn, \
                     tc.tile_pool(name="rtp", bufs=rpp_bufs, space="PSUM") as rpp, \
                     tc.tile_pool(name="rts", bufs=3) as rsp:
                    am_sb = rp.tile([128, KD * 2 * E], _F32, tag="am")
                    nc.sync.dma_start(
                        am_sb[:], AM.ap().rearrange("(k p) c -> p k c", p=128))
                    c_sb = rp.tile([1, E], _F32, tag="c")
                    nc.sync.dma_start(c_sb[:], crow[:, :])
                    xt_k, x2_k = [], []
                    xt_src = xT.ap().rearrange("(k p) t -> k p t", p=128)
                    for k in range(KD):
                        xt = rpk.tile([128, NTOK], _F32, tag="xt")
                        nc.sync.dma_start(xt[:], xt_src[k])
                        xt_k.append(xt)
                        if not (on_the_fly_sq or uniform_a):
                            x2 = rpk.tile([128, NTOK], _F32, tag="x2")
                            nc.scalar.activation(x2[:], xt[:], _AF.Square)
                            x2_k.append(x2)
                    if uniform_a:
                        a0_sb = rp.tile([128, 1], _F32, tag="a0")
                        nc.sync.dma_start(a0_sb[:], a0h[:, :])

                    for t0 in range(0, NTOK, 128):
                        pa = rpp.tile([128, E], _F32, tag="pa")
                        nc.tensor.matmul(pa[:], ones_f[:], c_sb[:],
                                         start=True, stop=False)
                        if uniform_a:
                            # sum(x^2)*a0 per token via ACT accumulate.
                            xn_sb = rxn.tile([128, D], _F32, tag="xn")
                            nc.sync.dma_start(xn_sb[:], xn[t0 : t0 + 128, :])
                            sq = rxn.tile([128, D], _F32, tag="sq")
                            s2 = rsp.tile([128, 1], _F32, tag="s2")
                            nc.scalar.activation(sq[:], xn_sb[:], _AF.Square,
                                                 accum_out=s2[:])
                            bn = rsp.tile([128, 1], _F32, tag="bn")
                            nc.vector.tensor_scalar_mul(bn[:], s2[:],
                                                        a0_sb[:])
                        else:
                            for k in range(KD):
                                if on_the_fly_sq:
                                    x2s = rsq.tile([128, 128], _F32, tag="x2s")
                                    nc.scalar.activation(
                                        x2s[:], xt_k[k][:, t0 : t0 + 128],
                                        _AF.Square)
                                    x2ap = x2s[:]
                                else:
                                    x2ap = x2_k[k][:, t0 : t0 + 128]
                                nc.tensor.matmul(
                                    pa[:], x2ap,
                                    am_sb[:, k * 2 * E : k * 2 * E + E],
                                    start=False, stop=False)
                        for k in range(KD):
                            nc.tensor.matmul(
                                pa[:],
                                xt_k[k][:, t0 : t0 + 128],
                                am_sb[:, k * 2 * E + E : (k + 1) * 2 * E],
                                start=False, stop=(k == KD - 1))
                        lp_sb = rsp.tile([128, E], _F32, tag="lp")
                        if uniform_a:
                            nc.scalar.activation(lp_sb[:], pa[:],
                                                 _AF.Identity,
                                                 bias=bn[:], scale=-0.5)
                        else:
                            nc.scalar.activation(lp_sb[:], pa[:], _AF.Copy,
                                                 bias=0.0, scale=-0.5)
                        nc.sync.dma_start(lp_out[t0 : t0 + 128, :], lp_sb[:])
                        nmax = rsp.tile([128, 1], _F32, tag="nmax")
                        nc.vector.tensor_reduce(nmax[:], lp_sb[:],
                                                axis=mybir.AxisListType.X,
                                                op=_ALU.max, negate=True)
                        esb = rsp.tile([128, E], _F32, tag="esb")
                        ssb = rsp.tile([128, 1], _F32, tag="ssb")
                        nc.scalar.activation(esb[:], lp_sb[:], _AF.Exp,
                                             bias=nmax[:], scale=1.0,
                                             accum_out=ssb[:])
                        rsb = rsp.tile([128, 1], _F32, tag="rsb")
                        nc.vector.reciprocal(rsb[:], ssb[:])
                        wsb = rsp.tile([128, E], _F32, tag="wsb")
                        nc.vector.tensor_scalar_mul(wsb[:], esb[:], rsb[:])
                        nc.sync.dma_start(w_out[t0 : t0 + 128, :], wsb[:])

            def emit_moe(tail=None, tail_seg=None):
                with tc.tile_pool(name="xg", bufs=xg_bufs) as xgp, \
                     tc.tile_pool(name="ht", bufs=ht_bufs) as htp, \
                     tc.tile_pool(name="w1p", bufs=w1_bufs) as w1p, \
                     tc.tile_pool(name="w2p", bufs=w2_bufs) as w2p, \
                     tc.tile_pool(name="sm", bufs=2) as smp, \
                     tc.tile_pool(name="ob", bufs=ob_bufs) as obp, \
                     tc.tile_pool(name="pA", bufs=abs(pa_bufs), space="PSUM") as pap, \
                     tc.tile_pool(name="pB", bufs=pb_bufs, space="PSUM") as _pbx:
                    pbp = pap if pa_bufs < 0 else _pbx
                    for seg in range(S):
                        T = Tlist[seg]
                        TC = T * 128
                        xg_sb = xgp.tile([128, KD * TC], _BF16, tag="xg")
                        nc.sync.dma_start(
                            xg_sb[:],
                            xgT.ap().rearrange("(k p) t -> p k t", p=128)
                            [:, :, seg_tok0[seg] : seg_tok0[seg] + TC])
                        b1_sb = smp.tile([128, MH], _F32, tag="b1")
                        nc.sync.dma_start(b1_sb[:], b1t[seg])
                        g_sb = smp.tile([128, T], _F32, tag="g")
                        nc.sync.dma_start(
                            g_sb[:], gT[:, seg_t0[seg] : seg_t0[seg] + T])

                        ht_sb = htp.tile([128, MH * TC], _BF16, tag="ht")

                        # phase A: ht[m, tok] = gelu(W1^T x + b1)
                        for m in range(MH):
                            w1m = w1p.tile([128, KD * 128], _BF16, tag="w1m")
                            nc.sync.dma_start(
                                w1m[:],
                                w1[seg, :, m].rearrange("k p q -> p k q"))
                            for boff, blen in _blocks(T):
                                ps = pap.tile([128, blen], _F32, tag="pA")
                                for k in range(KD):
                                    nc.tensor.matmul(
                                        ps[:],
                                        w1m[:, k * 128 : (k + 1) * 128],
                                        xg_sb[:, k * TC + boff : k * TC + boff + blen],
                                        start=(k == 0), stop=(k == KD - 1))
                                nc.scalar.activation(
                                    ht_sb[:, m * TC + boff : m * TC + boff + blen],
                                    ps[:], _AF.Gelu,
                                    bias=b1_sb[:, m : m + 1], scale=1.0)

                        # phase B: out[tok, n] = (ht^T W2 + b2) * gate
                        for n in range(NO):
                            w2t = []
                            for k in range(MH):
                                wt = w2p.tile([128, 512], _BF16, tag="w2t")
                                nc.sync.dma_start(wt[:], w2[seg, k, n])
                                w2t.append(wt)
                            for t in range(T):
                                ps = pbp.tile([128, 512], _F32, tag="pA" if pa_bufs < 0 else "pB")
                                for k in range(MH):
                                    nc.tensor.matmul(
                                        ps[:],
                                        ht_sb[:, k * TC + t * 128 : k * TC + t * 128 + 128],
                                        w2t[k][:],
                                        start=(k == 0), stop=(k == MH - 1))
                                osb = obp.tile([128, 512], _F32, tag="osb")
                                nc.vector.tensor_scalar_mul(
                                    osb[:], ps[:], g_sb[:, t : t + 1])
                                nc.sync.dma_start(
                                    out[(seg_t0[seg] + t) * 128 : (seg_t0[seg] + t + 1) * 128,
                                        n * 512 : (n + 1) * 512],
                                    osb[:])
                        if tail is not None and seg == tail_seg:
                            tail()
                    if tail is not None and tail_seg is None:
                        tail()

            if parts == "moe":
                emit_moe()
            elif parts == "routing":
                emit_routing()
            elif parts == "nested":
                emit_moe(tail=lambda: emit_routing(on_the_fly_sq=True),
                         tail_seg=tail_seg)
            elif routing_last:
                emit_moe()
                emit_routing()
            else:
                emit_routing()
                emit_moe()

    nc.finalize()
    _KERNEL_CACHE[key] = nc
    return nc


def _build_inputs(x_flat, W1, b1, W2, b2, expert_mus, expert_log_sigmas,
                  w_host, Tlist, slots, uniform_a=False):
    """Per-core input dicts + the (locA, locB) combine rows."""
    sig2 = np.exp(expert_log_sigmas).astype(np.float32) ** 2
    a = (1.0 / sig2).astype(np.float32)                      # [E, D]
    m2 = (-2.0 * expert_mus * a).astype(np.float32)          # [E, D]
    cvec = np.sum(expert_mus * expert_mus * a, -1) + 2.0 * np.sum(
        expert_log_sigmas, -1)
    AM = np.concatenate([a.T, m2.T], axis=1).astype(np.float32)  # [D, 2E]
    crow = cvec.reshape(1, E).astype(np.float32)

    S = len(Tlist)
    TT = sum(Tlist)
    ST = TT * 128
    seg_tok0 = [128 * sum(Tlist[:j]) for j in range(S)]
    seg_t0 = [sum(Tlist[:j]) for j in range(S)]
    xT_all = np.ascontiguousarray(x_flat.T)                  # [D, N]
    w1_bf = W1.astype(_BF_NP)
    w2_bf = W2.astype(_BF_NP)

    in_maps = []
    tok_all, row_all = [], []
    for c in range(NCORES):
        xgT = np.zeros((D, ST), _BF_NP)
        gTm = np.zeros((128, TT), np.float32)
        w1g = np.empty((S, KD, MH, 128, 128), _BF_NP)
        w2g = np.empty((S, MH, NO, 128, 512), _BF_NP)
        b1g = np.empty((S, 128, MH), np.float32)
        for s in range(S):
            T = Tlist[s]
            TC = T * 128
            e, toks = slots[s][c]
            nt = len(toks)
            if nt:
                xgT[:, seg_tok0[s] : seg_tok0[s] + nt] = \
                    xT_all[:, toks].astype(_BF_NP)
                g = w_host[toks, e]
                gfull = np.zeros(TC, np.float32)
                gfull[:nt] = g
                gTm[:, seg_t0[s] : seg_t0[s] + T] = gfull.reshape(T, 128).T
                row0 = c * ST + seg_tok0[s]
                tok_all.append(toks)
                row_all.append(row0 + np.arange(nt, dtype=np.int64))
            w1g[s] = (
                w1_bf[e]
                .reshape(KD, 128, MH, 128).transpose(0, 2, 1, 3))
            w2g[s] = (
                w2_bf[e]
                .reshape(MH, 128, NO, 512).transpose(0, 2, 1, 3))
            b1g[s] = b1[e].reshape(MH, 128).T
        in_map = {
            "xT": np.ascontiguousarray(xT_all[:, c * NTOK : (c + 1) * NTOK]),
            "AM": AM, "crow": crow,
            "xgT": xgT,
            "w1": np.ascontiguousarray(w1g),
            "w2": np.ascontiguousarray(w2g),
            "b1t": b1g, "gT": gTm,
        }
        if uniform_a:
            in_map["xn"] = np.ascontiguousarray(
                x_flat[c * NTOK : (c + 1) * NTOK])
            in_map["a0h"] = np.full((128, 1), -0.5 * a.flat[0], np.float32)
        in_maps.append(in_map)

    tok_all = np.concatenate(tok_all)
    row_all = np.concatenate(row_all)
    order = np.argsort(tok_all, kind="stable")
    assert (tok_all[order][0::2] == np.arange(N)).all()
    assert (tok_all[order][1::2] == np.arange(N)).all()
    locA = row_all[order[0::2]]
    locB = row_all[order[1::2]]
    return in_maps, locA, locB


def kernel(x, expert_mus, expert_log_sigmas, W1, b1, W2, b2):
    x = np.asarray(x, np.float32)
    expert_mus = np.asarray(expert_mus, np.float32)
    expert_log_sigmas = np.asarray(expert_log_sigmas, np.float32)
    W1 = np.asarray(W1, np.float32)
    b1 = np.asarray(b1, np.float32)
    W2 = np.asarray(W2, np.float32)
    b2 = np.asarray(b2, np.float32)

    x_flat = np.ascontiguousarray(x.reshape(N, D))
    lp_host, w_host, ti = _host_routing(x_flat, expert_mus, expert_log_sigmas)
    Tlist, slots = _plan_slots(ti)
    # Uniform-sigma fast path: x^2 routing term reduces to a per-token
    # scalar computable on the ACT engine, halving routing matmuls.
    a_all = np.exp(-2.0 * expert_log_sigmas).astype(np.float32)
    uniform_a = bool((a_all == a_all.flat[0]).all())
    # Adaptive SBUF budget (KB/partition, ~186 usable): ht=8*Tmax,
    # xg=2*Tmax per buf, w1=2 per buf, w2=1 per buf, ob=8, routing~36-52.
    Tmax = max(Tlist)
    routing_kb = 50 if uniform_a else 36
    w2_nested = int(min(52, 196 - routing_kb - 10 * Tmax - 18))
    xg_bufs = 2 if 8 * Tmax + 2 * 2 * Tmax + 80 <= 186 else 1
    w2_bufs = int(min(56, max(8, 186 - 8 * Tmax - xg_bufs * 2 * Tmax - 25)))
    cfgs = []
    if w2_nested >= 34:
        # Routing overlaps the MoE phase (shares SBUF budget with it).
        cfgs.append(dict(parts="nested", w1_bufs=6, w2_bufs=w2_nested,
                         xg_bufs=1, uniform_a=uniform_a))
        cfgs.append(dict(parts="nested", w1_bufs=4,
                         w2_bufs=max(34, w2_nested - 8),
                         xg_bufs=1, uniform_a=uniform_a))
    cfgs.append(dict(routing_last=True,
                     w1_bufs=6 if w2_bufs >= 24 else 3, w2_bufs=w2_bufs,
                     xg_bufs=xg_bufs, uniform_a=uniform_a))
    nc = None
    for cfg in cfgs:
        try:
            nc = _build_kernel(Tlist, **cfg)
            break
        except Exception:  # noqa: BLE001 - SBUF/PSUM overflow: try smaller
            if cfg is cfgs[-1]:
                raise
    in_maps, locA, locB = _build_inputs(
        x_flat, W1, b1, W2, b2, expert_mus, expert_log_sigmas,
        w_host, Tlist, slots, uniform_a=uniform_a)

    res = None
    for attempt in range(3):
        try:
            res = run_bass_kernel_spmd(nc, in_maps,
                                       core_ids=list(range(NCORES)))
            break
        except Exception:  # noqa: BLE001 - transient NRT/device wedges
            if attempt == 2:
                raise
            time.sleep(5.0)

    lp = np.concatenate([r["lp_out"] for r in res.results]).reshape(B, TSEQ, E)
    w = np.concatenate([r["w_out"] for r in res.results]).reshape(B, TSEQ, E)
    outcat = np.concatenate([r["out"] for r in res.results], axis=0)
    # The device computes gate * (h @ W2); the gate * b2 term is added here.
    gsel = np.take_along_axis(w_host, ti, axis=1)            # [N, K]
    bias = np.einsum("nk,nkd->nd", gsel, b2[ti])             # [N, DO]
    final = (outcat[locA] + outcat[locB] + bias).reshape(B, TSEQ, DO)
    routing_info = {"log_probs": lp, "weights": w, "top_indices": ti}
    return final, routing_info
